# revision 58
# baseline (speedup 1.0000x reference)
"""Trainium2 Bass kernel for nn_MemPIDModel (dense_cnn).

Strategy (8 NeuronCores, no collectives):
  - core c handles sample b = c//4 (trunk replicated within each 4-core group)
  - core c computes vocab shard v = c%4 of the tied head: [1024,512]@[512,8000]
  - trunk in "Layout A": activations kept as x^T [D=512 partitions (4 tiles), T=1024 free]
  - matmuls + conv in bf16 (fp32 PSUM accumulation), residual stream in fp32
  - depthwise dilated conv on the TENSOR engine: per-tap diagonalized [128,128]
    weight blocks, 15 taps PSUM-accumulated per (dtile, 512-chunk); host
    pre-diagonalizes taps (norm weight folded in)
  - rmsnorm: square (DVE) -> PE matmul partition-reduce (gnorm^2 folded into
    lhsT for PID norms) -> exp(-0.5*ln(m+eps)) on ACT -> gpsimd broadcast
  - PID gate cross-layer pipelined: zb(li+1) = ki'*integ + (kp+ki')*x computed
    at layer li's tail (TS during SwiGLU, STT interleaved with the residual),
    so layer li+1 starts directly at its silu; integral update on DVE
    (gpsimd full-tile ops stall DVE via shared SBUF ports)
  - logits emitted bf16 (halves the 32MB output DMA)
"""

import os
import sys
import numpy as np

sys.path.insert(0, "/opt/trn_rl_repo")

import ml_dtypes

B = 2
T = 1024
D = 512
HID = 1024
KK = 15
VOCAB = 32000
RANK = 64
NL = 6
MIX_W = 0.1
UP_DIL = [1, 2, 4, 8, 16, 32]
DN_DIL = UP_DIL[::-1]
EPS = 1e-6
NCORES = 8
VSHARDS = 4
VS = VOCAB // VSHARDS  # 8000
DT = D // 128  # 4 D-tiles
HT = HID // 128  # 8 H-tiles
TT2 = T // 512  # 2 T-chunks of 512 for psum
PAD = 448  # (K-1)*max_dil
CONVW = PAD + T  # padded conv input width per d-tile

BF16 = ml_dtypes.bfloat16

# debug knobs (affect program shape; kernel cache key includes them)
N_LAYERS = int(os.environ.get("KB_LAYERS", str(NL)))
N_STACKS = int(os.environ.get("KB_STACKS", "3"))
SKIP_HEAD = bool(int(os.environ.get("KB_SKIP_HEAD", "0")))
DEBUG_TRUNK_OUT = bool(int(os.environ.get("KB_TRUNK_OUT", "0")))
PROBE_LI = int(os.environ.get("KB_PROBE_LI", "-1"))
PROBE_WHAT = os.environ.get("KB_PROBE_WHAT", "ub")

_prog_cache = {}


def _f32(x):
    return np.ascontiguousarray(np.asarray(x), dtype=np.float32)


def _bf(x):
    return np.ascontiguousarray(np.asarray(x).astype(np.float32), dtype=BF16)


def _cols(v):
    """[D] vector -> [128, DT] A-layout per-partition columns."""
    return np.ascontiguousarray(_f32(v).reshape(DT, 128).T)


def _stack_dils(stack_idx):
    return UP_DIL if stack_idx in (0, 2) else DN_DIL


def build_program():
    key = (N_LAYERS, N_STACKS, SKIP_HEAD, DEBUG_TRUNK_OUT, PROBE_LI, PROBE_WHAT)
    if key in _prog_cache:
        return _prog_cache[key]

    import concourse.bass as bass
    import concourse.mybir as mybir
    import concourse.tile as tile
    from concourse import bacc
    from concourse.masks import make_identity

    dt = mybir.dt
    Alu = mybir.AluOpType
    Act = mybir.ActivationFunctionType

    nc = bacc.Bacc(None, target_bir_lowering=False, debug=False)

    # ---------------- DRAM I/O ----------------
    d_idx = nc.dram_tensor("idx_rs", [128, 8], dt.int32, kind="ExternalInput")
    d_emb = nc.dram_tensor("emb_tbl", [VOCAB, D], dt.float32, kind="ExternalInput")
    d_pos = nc.dram_tensor("pos_rs", [128, 8 * D], dt.bfloat16, kind="ExternalInput")
    d_cst = nc.dram_tensor("cst", [128, 64], dt.float32, kind="ExternalInput")
    d_rc = nc.dram_tensor("rc_bc", [128, T], dt.bfloat16, kind="ExternalInput")
    d_mgw = nc.dram_tensor("mgwT", [D, D], dt.bfloat16, kind="ExternalInput")

    d_cw = {}
    d_cwd = {}
    d_w13 = {}
    d_w2 = {}
    d_pid = {}
    d_g2 = {}
    for s in ("up", "dn"):
        d_cw[s] = nc.dram_tensor(
            f"{s}_cw", [NL, 128, DT * 16], dt.float32, kind="ExternalInput"
        )
        d_cwd[s] = nc.dram_tensor(
            f"{s}_cwd", [NL, 128, DT * KK * 128], dt.bfloat16, kind="ExternalInput"
        )
        # w13: [NL, D, 2*HID] : per K-row, w1T cols then w3T cols
        d_w13[s] = nc.dram_tensor(
            f"{s}_w13T", [NL, D, 2 * HID], dt.bfloat16, kind="ExternalInput"
        )
        d_w2[s] = nc.dram_tensor(f"{s}_w2T", [NL, HID, D], dt.bfloat16, kind="ExternalInput")
        d_pid[s] = nc.dram_tensor(
            f"{s}_pid", [128, (NL - 1) * 12], dt.float32, kind="ExternalInput"
        )
        d_g2[s] = nc.dram_tensor(
            f"{s}_g2", [128, (NL - 1) * DT], dt.bfloat16, kind="ExternalInput"
        )
    d_dwT = nc.dram_tensor("sg_dwT", [3, D, RANK], dt.bfloat16, kind="ExternalInput")
    d_uwT = nc.dram_tensor("sg_uwT", [3, RANK, D], dt.bfloat16, kind="ExternalInput")
    d_sgc = nc.dram_tensor("sg_cols", [128, 16], dt.float32, kind="ExternalInput")
    d_embT = nc.dram_tensor("embT_sh", [D, VS], dt.bfloat16, kind="ExternalInput")

    d_out = nc.dram_tensor("logits_sh", [T, VS], dt.bfloat16, kind="ExternalOutput")
    if DEBUG_TRUNK_OUT:
        d_trunk = nc.dram_tensor("trunk_out", [128, DT * T], dt.float32, kind="ExternalOutput")

    f32 = dt.float32
    bf = dt.bfloat16

    with tile.TileContext(nc) as tc:
        import contextlib

        ctx = contextlib.ExitStack()
        with ctx:
            const = ctx.enter_context(tc.tile_pool(name="const", bufs=1))
            master = ctx.enter_context(tc.tile_pool(name="master", bufs=1))
            lay = ctx.enter_context(tc.tile_pool(name="lay", bufs=1))
            wgt = ctx.enter_context(tc.tile_pool(name="wgt", bufs=2))
            psum = ctx.enter_context(tc.tile_pool(name="psum", bufs=1, space="PSUM"))

            # ---------------- constants ----------------
            epsc = const.tile([128, 1], f32, tag="epsc")
            nc.vector.memset(epsc[:], EPS)
            ones_bf = const.tile([128, 1], bf, tag="ones")
            nc.vector.memset(ones_bf[:], 1.0)
            ident = const.tile([128, 128], f32, tag="ident")
            make_identity(nc, ident[:])
            cst = const.tile([128, 64], f32, tag="cst")
            nc.sync.dma_start(cst[:], d_cst[:])
            rc_bc = const.tile([128, T], bf, tag="rc")
            nc.sync.dma_start(rc_bc[:], d_rc[:])
            sgc = const.tile([128, 16], f32, tag="sgc")
            nc.sync.dma_start(sgc[:], d_sgc[:])
            pidc = {}
            g2c = {}
            for s in ("up", "dn"):
                pidc[s] = const.tile(
                    [128, (NL - 1) * 12], f32, tag=f"pid_{s}", name=f"pid_{s}"
                )
                nc.sync.dma_start(pidc[s][:], d_pid[s][:])
                g2c[s] = const.tile(
                    [128, (NL - 1) * DT], bf, tag=f"g2_{s}", name=f"g2_{s}"
                )
                nc.sync.dma_start(g2c[s][:], d_g2[s][:])

            # persistent activations (A-layout, free index = dt*T + t)
            xA = master.tile([128, DT * T], f32, tag="xA")
            # next-layer PID pre-activation, written at each layer's tail
            zbt = master.tile([128, DT * T], bf, tag="zbt")

            def keep_tile():  # initial, then gated2 (sequential lifetimes)
                return master.tile([128, DT * T], f32, tag="keep", name="keep")

            def f32a_tile():  # integ during stacks / mixed during boundaries
                return lay.tile([128, DT * T], f32, tag="f32a", name="f32a")

            def wA():
                return lay.tile([128, DT * T], bf, tag="wA", name="wA")

            def wB():
                return lay.tile([128, DT * T], bf, tag="wB", name="wB")

            def wC():
                return lay.tile([128, DT * T], bf, tag="wC", name="wC")

            # ---------------- helpers ----------------
            def norm_scale_row(src_bf, sq, wcol=None):
                """src/sq: [128, DT*T] bf16. Returns s_bc [128,T] bf16 tile.

                wcol(kt) optionally supplies a per-channel-weighted lhsT
                column (e.g. gnorm^2) in place of ones for the reduce.
                sq emitted per-dtile so the kt-chained reduce matmuls start
                as soon as their k-tile's square lands.
                """
                for kt in range(DT):
                    nc.vector.tensor_tensor(
                        out=sq[:, kt * T : (kt + 1) * T],
                        in0=src_bf[:, kt * T : (kt + 1) * T],
                        in1=src_bf[:, kt * T : (kt + 1) * T],
                        op=Alu.mult,
                    )
                s_row = lay.tile([128, T], bf, tag="srow", name="srow")
                lrow = lay.tile([128, 512], f32, tag="lrow", name="lrow", bufs=1)
                for nt in range(TT2):
                    ps = psum.tile([128, 512], f32, tag=f"ps_y{nt}", bufs=1, name="ps_norm")
                    for kt in range(DT):
                        nc.tensor.matmul(
                            ps[0:1, :],
                            lhsT=(ones_bf[:] if wcol is None else wcol(kt)),
                            rhs=sq[:, kt * T + nt * 512 : kt * T + (nt + 1) * 512],
                            start=(kt == 0),
                            stop=(kt == DT - 1),
                        )
                    nc.scalar.activation(
                        lrow[0:1, :], ps[0:1, :], Act.Ln, bias=epsc[0:1, :], scale=1.0 / D
                    )
                    nc.scalar.activation(
                        s_row[0:1, nt * 512 : (nt + 1) * 512], lrow[0:1, :], Act.Exp, scale=-0.5
                    )
                s_bc = lay.tile([128, T], bf, tag="sbc", name="sbc")
                for nt in range(TT2):
                    nc.gpsimd.partition_broadcast(
                        s_bc[:, nt * 512 : (nt + 1) * 512],
                        s_row[0:1, nt * 512 : (nt + 1) * 512],
                    )
                return s_bc

            # ---------------- P0: gather + embnorm + shift + mem ----------------
            with tc.tile_pool(name="p0", bufs=1) as p0:
                idx_sb = p0.tile([128, 8], dt.int32, tag="idx")
                nc.sync.dma_start(idx_sb[:], d_idx[:])
                gth = p0.tile([128, 8 * D], f32, tag="gth")
                for c in range(8):
                    nc.gpsimd.indirect_dma_start(
                        out=gth[:, c * D : (c + 1) * D],
                        out_offset=None,
                        in_=d_emb[:],
                        in_offset=bass.IndirectOffsetOnAxis(ap=idx_sb[:, c : c + 1], axis=0),
                    )
                for h in range(2):
                    pos_sb = p0.tile([128, 4 * D], bf, tag="pos", bufs=1)
                    nc.sync.dma_start(pos_sb[:], d_pos[:, h * 4 * D : (h + 1) * 4 * D])
                    nc.vector.tensor_tensor(
                        out=gth[:, h * 4 * D : (h + 1) * 4 * D],
                        in0=gth[:, h * 4 * D : (h + 1) * 4 * D],
                        in1=pos_sb[:],
                        op=Alu.add,
                    )
                ss = p0.tile([128, 8], f32, tag="ss")
                sqt = p0.tile([128, D], f32, tag="sqt")
                for c in range(8):
                    nc.scalar.activation(
                        sqt[:],
                        gth[:, c * D : (c + 1) * D],
                        Act.Square,
                        accum_out=ss[:, c : c + 1],
                    )
                nc.scalar.activation(ss[:], ss[:], Act.Ln, bias=epsc[:], scale=1.0 / D)
                nc.scalar.activation(ss[:], ss[:], Act.Exp, scale=-0.5)
                for c in range(8):
                    nc.vector.tensor_scalar(
                        gth[:, c * D : (c + 1) * D],
                        gth[:, c * D : (c + 1) * D],
                        ss[:, c : c + 1],
                        None,
                        Alu.mult,
                    )
                # transpose B->A
                x_n = p0.tile([128, DT * T], f32, tag="xn_a")
                for c in range(8):
                    pst = psum.tile([128, 512], f32, tag="ps_y2", bufs=1, name="ps_tp")
                    for dtt in range(DT):
                        nc.tensor.transpose(
                            out=pst[:, dtt * 128 : (dtt + 1) * 128],
                            in_=gth[:, c * D + dtt * 128 : c * D + (dtt + 1) * 128],
                            identity=ident[:],
                        )
                    for dtt in range(DT):
                        nc.vector.tensor_copy(
                            x_n[:, dtt * T + c * 128 : dtt * T + (c + 1) * 128],
                            pst[:, dtt * 128 : (dtt + 1) * 128],
                        )
                # mem gate (kt-outer so one [128,D] weight chunk is live at a time)
                cstb = p0.tile([128, 4], bf, tag="cstb")
                nc.vector.tensor_copy(cstb[:], cst[:, 16:20])
                ps_mem = psum.tile([128, 4], f32, tag="ps_y3", bufs=1, name="ps_mem")
                for kt in range(DT):
                    mgw_sb = p0.tile([128, D], bf, tag="mgwb", name="mgw_sb", bufs=2)
                    nc.sync.dma_start(mgw_sb[:], d_mgw[kt * 128 : (kt + 1) * 128, :])
                    for m in range(DT):
                        nc.tensor.matmul(
                            ps_mem[:, m : m + 1],
                            lhsT=mgw_sb[:, m * 128 : (m + 1) * 128],
                            rhs=cstb[:, kt : kt + 1],
                            start=(kt == 0),
                            stop=(kt == DT - 1),
                        )
                tmem = p0.tile([128, 4], f32, tag="tmem")
                for m in range(DT):
                    nc.scalar.activation(
                        tmem[:, m : m + 1],
                        ps_mem[:, m : m + 1],
                        Act.Tanh,
                        scale=0.5,
                        bias=cst[:, 12 + m : 13 + m],
                    )
                nc.vector.tensor_scalar(tmem[:], tmem[:], 0.5, 0.5, Alu.mult, Alu.add)
                # token shift + mem (in-place: xA = (1-ts)x[t] then += ts*x[t-1])
                for dtt in range(DT):
                    o = dtt * T
                    nc.vector.tensor_scalar(
                        xA[:, o : o + 1], x_n[:, o : o + 1], cst[:, dtt : dtt + 1], None, Alu.mult
                    )
                    nc.vector.tensor_scalar(
                        xA[:, o + 1 : o + T],
                        x_n[:, o + 1 : o + T],
                        cst[:, 8 + dtt : 9 + dtt],
                        None,
                        Alu.mult,
                    )
                    nc.vector.scalar_tensor_tensor(
                        out=xA[:, o + 1 : o + T],
                        in0=x_n[:, o : o + T - 1],
                        scalar=cst[:, 4 + dtt : 5 + dtt],
                        in1=xA[:, o + 1 : o + T],
                        op0=Alu.mult,
                        op1=Alu.add,
                    )
                    nc.vector.tensor_scalar(
                        xA[:, o : o + T], xA[:, o : o + T], tmem[:, dtt : dtt + 1], None, Alu.add
                    )
            initial = keep_tile()
            nc.vector.tensor_scalar(initial[:], xA[:], 1.0, None, Alu.mult)

            # ---------------- conv block stack ----------------
            def run_stack(stack_idx):
                s = "up" if stack_idx in (0, 2) else "dn"
                dils = _stack_dils(stack_idx)
                integ = f32a_tile()  # initialized off-path inside layer 0
                for li in range(N_LAYERS):
                    d = dils[li]
                    cw_sb = wgt.tile([128, DT * 16], f32, tag="cw", name="cw")
                    nc.sync.dma_start(cw_sb[:], d_cw[s][li])
                    cwd_sb = wgt.tile(
                        [128, DT * KK * 128], bf, tag="cwd", name="cwd", bufs=1
                    )
                    nc.sync.dma_start(cwd_sb[:], d_cwd[s][li])
                    w13_sb = wgt.tile([128, DT * 2 * HID], bf, tag="w13", name="w13", bufs=1)
                    for kt in range(DT):
                        nc.sync.dma_start(
                            w13_sb[:, kt * 2 * HID : (kt + 1) * 2 * HID],
                            d_w13[s][li, kt * 128 : (kt + 1) * 128, :],
                        )
                    w2_sb = wgt.tile([128, HT * D], bf, tag="w2", name="w2", bufs=1)
                    for kh in range(HT):
                        nc.sync.dma_start(
                            w2_sb[:, kh * D : (kh + 1) * D],
                            d_w2[s][li, kh * 128 : (kh + 1) * 128, :],
                        )

                    # ---- PID gate (li>0): zb = ki'*integ_old + (kp+ki')*xA
                    # was computed at the tail of the PREVIOUS layer (TS
                    # during its SwiGLU; STT interleaved with its residual),
                    # so this layer starts directly at the silu.
                    sq = wA()  # holds zb from prev layer; reused as sq post-silu
                    ub = wB()
                    if DEBUG_TRUNK_OUT and PROBE_LI == li and PROBE_WHAT in ("integ", "cur"):
                        probe = lay.tile([128, DT * T], f32, tag="probe", name="probe")
                        srct = integ if PROBE_WHAT == "integ" else xA
                        nc.vector.tensor_scalar(probe[:], srct[:], 1.0, None, Alu.mult)
                        nc.sync.dma_start(d_trunk[:], probe[:])
                    if li > 0:
                        pc = pidc[s]
                        pbase = (li - 1) * 12
                        zb = zbt
                        if DEBUG_TRUNK_OUT and PROBE_LI == li and PROBE_WHAT == "zb":
                            probe = lay.tile([128, DT * T], f32, tag="probe", name="probe")
                            nc.vector.tensor_scalar(probe[:], zb[:], 1.0, None, Alu.mult)
                            nc.sync.dma_start(d_trunk[:], probe[:])
                        for dtt in range(DT):
                            nc.scalar.activation(
                                ub[:, dtt * T : (dtt + 1) * T],
                                zb[:, dtt * T : (dtt + 1) * T],
                                Act.Silu,
                            )
                        # norm reduce uses gnorm^2-weighted lhsT on pre-gnorm
                        # silu, so the gnorm scale below runs in its shadow
                        s_bc = norm_scale_row(
                            ub, sq,
                            wcol=lambda kt: g2c[s][:, (li - 1) * DT + kt : (li - 1) * DT + kt + 1],
                        )
                        for dtt in range(DT):
                            nc.vector.tensor_scalar(
                                ub[:, dtt * T : (dtt + 1) * T],
                                ub[:, dtt * T : (dtt + 1) * T],
                                pc[:, pbase + 8 + dtt : pbase + 9 + dtt],
                                None,
                                Alu.mult,
                            )
                    else:
                        ub = xA  # norm + xnb read xA directly (f32, 2x_2p)
                        s_bc = norm_scale_row(xA, sq)

                    if DEBUG_TRUNK_OUT and PROBE_LI == li and PROBE_WHAT == "ub":
                        probe = lay.tile([128, DT * T], f32, tag="probe", name="probe")
                        nc.vector.tensor_scalar(probe[:], ub[:], 1.0, None, Alu.mult)
                        nc.sync.dma_start(d_trunk[:], probe[:])

                    # ---- normed into conv pad buffer
                    xnb = lay.tile([128, DT * CONVW], bf, tag="xnb", name="xnb")
                    for dtt in range(DT):
                        nc.gpsimd.memset(
                            xnb[:, dtt * CONVW + PAD - 14 * d : dtt * CONVW + PAD], 0.0
                        )
                    for nt in range(TT2):
                        for dtt in range(DT):
                            ob = dtt * CONVW
                            nc.vector.tensor_tensor(
                                out=xnb[:, ob + PAD + nt * 512 : ob + PAD + (nt + 1) * 512],
                                in0=ub[:, dtt * T + nt * 512 : dtt * T + (nt + 1) * 512],
                                in1=s_bc[:, nt * 512 : (nt + 1) * 512],
                                op=Alu.mult,
                            )
                    if li == 0:
                        # integral_0 = stack input; deferred here (off the
                        # layer-0 norm critical path), first read at this
                        # layer's SwiGLU tail
                        nc.vector.tensor_scalar(integ[:], xA[:], 1.0, None, Alu.mult)
                    elif li < N_LAYERS - 1:
                        # integral += prev layer's output. DVE, not gpsimd:
                        # shared SBUF ports stall concurrent DVE ops.
                        nc.vector.tensor_tensor(
                            out=integ[:], in0=integ[:], in1=xA[:], op=Alu.add
                        )

                    # ---- depthwise causal dilated conv on PE (diag-block
                    # matmuls accumulating 15 taps in PSUM) + bias + silu
                    hb = wC()
                    for dtt in range(DT):
                        ob = dtt * CONVW
                        oa = dtt * T
                        wb_ = dtt * 16
                        for nt in range(TT2):
                            pcv = psum.tile(
                                [128, 512], f32, tag="ps_g", bufs=2, name="ps_cv"
                            )
                            for mi, m in enumerate(range(KK - 1, -1, -1)):
                                st = ob + PAD + nt * 512 - m * d
                                nc.tensor.matmul(
                                    pcv[:],
                                    lhsT=cwd_sb[
                                        :, (dtt * KK + m) * 128 : (dtt * KK + m + 1) * 128
                                    ],
                                    rhs=xnb[:, st : st + 512],
                                    start=(mi == 0),
                                    stop=(mi == KK - 1),
                                )
                            nc.scalar.activation(
                                hb[:, oa + nt * 512 : oa + (nt + 1) * 512],
                                pcv[:],
                                Act.Silu,
                                bias=cw_sb[:, wb_ + 15 : wb_ + 16],
                            )

                    # ---- SwiGLU + W2, chunked over (nt, kh) to bound SBUF/PSUM
                    for nt in range(TT2):
                        psy = [
                            psum.tile([128, 512], f32, tag=f"ps_y{md}", bufs=1, name=f"ps_y{md}")
                            for md in range(DT)
                        ]
                        for kh in range(HT):
                            gs_t = lay.tile([128, 512], bf, tag="gs_t", name="gs_t", bufs=1)
                            pch = lay.tile([128, 512], bf, tag="pch", name="pch", bufs=2)
                            upch = lay.tile([128, 512], bf, tag="upch", name="upch", bufs=2)
                            psg = psum.tile([128, 512], f32, tag="ps_g", bufs=2, name="ps_g")
                            for kt in range(DT):
                                nc.tensor.matmul(
                                    psg[:],
                                    lhsT=w13_sb[:, kt * 2 * HID + kh * 128 : kt * 2 * HID + (kh + 1) * 128],
                                    rhs=hb[:, kt * T + nt * 512 : kt * T + (nt + 1) * 512],
                                    start=(kt == 0),
                                    stop=(kt == DT - 1),
                                )
                            nc.scalar.activation(gs_t[:], psg[:], Act.Silu)
                            psu = psum.tile([128, 512], f32, tag="ps_g", bufs=2, name="ps_u")
                            for kt in range(DT):
                                nc.tensor.matmul(
                                    psu[:],
                                    lhsT=w13_sb[:, kt * 2 * HID + HID + kh * 128 : kt * 2 * HID + HID + (kh + 1) * 128],
                                    rhs=hb[:, kt * T + nt * 512 : kt * T + (nt + 1) * 512],
                                    start=(kt == 0),
                                    stop=(kt == DT - 1),
                                )
                            nc.scalar.activation(upch[:], psu[:], Act.Copy)
                            nc.vector.tensor_tensor(out=pch[:], in0=gs_t[:], in1=upch[:], op=Alu.mult)
                            for md in range(DT):
                                nc.tensor.matmul(
                                    psy[md][:],
                                    lhsT=w2_sb[:, kh * D + md * 128 : kh * D + (md + 1) * 128],
                                    rhs=pch[:],
                                    start=(kh == 0),
                                    stop=(kh == HT - 1),
                                )
                        if nt == 1 and li < N_LAYERS - 1:
                            # next layer's PID prep: zb_next = ki'*integ now
                            # (fills DVE while PE finishes psy), then the
                            # (kp+ki')*xA term interleaved per-dtile with the
                            # residual adds below
                            zbn = zbt
                            pcn = pidc[s]
                            pn = li * 12
                            for dtt in range(DT):
                                nc.vector.tensor_scalar(
                                    zbn[:, dtt * T : (dtt + 1) * T],
                                    integ[:, dtt * T : (dtt + 1) * T],
                                    pcn[:, pn + 4 + dtt : pn + 5 + dtt],
                                    None,
                                    Alu.mult,
                                )
                        for md in range(DT):
                            xs = xA[:, md * T + nt * 512 : md * T + (nt + 1) * 512]
                            if li > 0:
                                # residual base is cur_in (the PID-gated normed
                                # tensor, = xnb contents), NOT the prev layer out
                                base = xnb[:, md * CONVW + PAD + nt * 512 : md * CONVW + PAD + (nt + 1) * 512]
                            else:
                                base = xs
                            nc.vector.tensor_tensor(out=xs, in0=base, in1=psy[md][:], op=Alu.add)
                            if nt == 1 and li < N_LAYERS - 1:
                                o = md * T
                                nc.vector.scalar_tensor_tensor(
                                    out=zbn[:, o : o + T],
                                    in0=xA[:, o : o + T],
                                    scalar=pcn[:, pn + md : pn + md + 1],
                                    in1=zbn[:, o : o + T],
                                    op0=Alu.mult,
                                    op1=Alu.add,
                                )

            # ---------------- mix + sgate boundary ----------------
            def boundary(k, old_tile):
                mixed = f32a_tile()  # integ dead
                cs = wC()
                # per-dtile scan -> rc-scale -> add so the norm reduce below
                # starts after dtile 0 lands instead of after all four
                for dtt in range(DT):
                    o = dtt * T
                    nc.vector.tensor_tensor_scan(
                        out=cs[:, o : o + T],
                        data0=xA[:, o : o + T],
                        data1=xA[:, o : o + T],
                        initial=0.0,
                        op0=Alu.add,
                        op1=Alu.bypass,
                    )
                    nc.vector.tensor_tensor(
                        out=cs[:, o : o + T], in0=cs[:, o : o + T], in1=rc_bc[:], op=Alu.mult
                    )
                    nc.vector.tensor_tensor(
                        out=mixed[:, o : o + T], in0=xA[:, o : o + T],
                        in1=cs[:, o : o + T], op=Alu.add
                    )

                sq = wB()
                s_bc = norm_scale_row(mixed, sq)
                nb = wB()  # sq dead
                for dtt in range(DT):
                    nc.vector.tensor_tensor(
                        out=nb[:, dtt * T : (dtt + 1) * T],
                        in0=mixed[:, dtt * T : (dtt + 1) * T],
                        in1=s_bc[:],
                        op=Alu.mult,
                    )
                dw_sb = wgt.tile([128, DT * RANK], bf, tag="dw", name="dw")
                for kt in range(DT):
                    nc.sync.dma_start(
                        dw_sb[:, kt * RANK : (kt + 1) * RANK],
                        d_dwT[k, kt * 128 : (kt + 1) * 128, :],
                    )
                uw_sb = wgt.tile([128, D], bf, tag="uw", name="uw")
                nc.sync.dma_start(uw_sb[0:RANK, :], d_uwT[k])
                hsb = lay.tile([128, T], bf, tag="hsb", name="hsb")
                for nt in range(TT2):
                    psh = psum.tile([128, 512], f32, tag="ps_g", bufs=2, name="ps_h")
                    for kt in range(DT):
                        nc.tensor.matmul(
                            psh[0:RANK, :],
                            lhsT=dw_sb[:, kt * RANK : (kt + 1) * RANK],
                            rhs=nb[:, kt * T + nt * 512 : kt * T + (nt + 1) * 512],
                            start=(kt == 0),
                            stop=(kt == DT - 1),
                        )
                    nc.scalar.activation(
                        hsb[0:RANK, nt * 512 : (nt + 1) * 512],
                        psh[0:RANK, :],
                        Act.Silu,
                        bias=sgc[0:RANK, k : k + 1],
                    )
                # blend per dtile: xA = old + (0.5 + 0.5*t) * (new - old),
                # DVE blend of dtile md overlapping the gate matmuls of md+1
                tg = wC()
                df = wA()
                for md in range(DT):
                    o = md * T
                    for nt in range(TT2):
                        psg2 = psum.tile([128, 512], f32, tag="ps_g", bufs=2, name="ps_g2")
                        nc.tensor.matmul(
                            psg2[:],
                            lhsT=uw_sb[0:RANK, md * 128 : (md + 1) * 128],
                            rhs=hsb[0:RANK, nt * 512 : (nt + 1) * 512],
                            start=True,
                            stop=True,
                        )
                        nc.scalar.activation(
                            tg[:, o + nt * 512 : o + (nt + 1) * 512],
                            psg2[:],
                            Act.Tanh,
                            scale=0.5,
                            bias=sgc[:, 4 + k * 4 + md : 5 + k * 4 + md],
                        )
                    nc.vector.tensor_tensor(
                        out=df[:, o : o + T], in0=mixed[:, o : o + T],
                        in1=old_tile[:, o : o + T], op=Alu.subtract)
                    nc.vector.tensor_scalar(
                        tg[:, o : o + T], tg[:, o : o + T], 0.5, 0.5, Alu.mult, Alu.add)
                    nc.vector.tensor_tensor(
                        out=df[:, o : o + T], in0=tg[:, o : o + T],
                        in1=df[:, o : o + T], op=Alu.mult)
                    nc.vector.tensor_tensor(
                        out=xA[:, o : o + T], in0=old_tile[:, o : o + T],
                        in1=df[:, o : o + T], op=Alu.add)

            # ---------------- run the model ----------------
            gated2 = None
            for si in range(N_STACKS):
                run_stack([0, 1, 2][si])
                if si == 0:
                    boundary(0, initial)
                elif si == 1:
                    boundary(1, initial)
                    gated2 = keep_tile()  # initial dead
                    nc.vector.tensor_scalar(gated2[:], xA[:], 1.0, None, Alu.mult)
                elif si == 2:
                    boundary(2, gated2)

            if DEBUG_TRUNK_OUT:
                nc.sync.dma_start(d_trunk[:], xA[:])

            # ---------------- final rmsnorm + tied head ----------------
            if not SKIP_HEAD:
                ob = wA()
                nc.vector.tensor_scalar(ob[:], xA[:], 1.0, None, Alu.mult)
                sq = wB()
                nc.vector.tensor_tensor(out=sq[:], in0=ob[:], in1=ob[:], op=Alu.mult)
                s_row = lay.tile([128, T], f32, tag="srow_f", name="srow_f")
                for nt in range(TT2):
                    ps = psum.tile([128, 512], f32, tag=f"ps_y{nt}", bufs=1, name="ps_n2")
                    for kt in range(DT):
                        nc.tensor.matmul(
                            ps[0:1, :],
                            lhsT=ones_bf[:],
                            rhs=sq[:, kt * T + nt * 512 : kt * T + (nt + 1) * 512],
                            start=(kt == 0),
                            stop=(kt == DT - 1),
                        )
                    nc.scalar.activation(
                        s_row[0:1, nt * 512 : (nt + 1) * 512],
                        ps[0:1, :],
                        Act.Ln,
                        bias=epsc[0:1, :],
                        scale=1.0 / D,
                    )
                nc.scalar.activation(s_row[0:1, :], s_row[0:1, :], Act.Exp, scale=-0.5)
                s_colT = lay.tile([128, 8], f32, tag="scolT", name="scolT")
                for g in range(2):
                    pst = psum.tile([128, 512], f32, tag="ps_y2", bufs=1, name="ps_tp2")
                    for c in range(4):
                        nc.tensor.transpose(
                            out=pst[:, c * 128 : c * 128 + 1],
                            in_=s_row[0:1, (g * 4 + c) * 128 : (g * 4 + c + 1) * 128],
                            identity=ident[0:1, 0:1],
                        )
                    for c in range(4):
                        nc.vector.tensor_copy(
                            s_colT[:, g * 4 + c : g * 4 + c + 1], pst[:, c * 128 : c * 128 + 1]
                        )
                NV = (VS + 511) // 512
                for nv in range(NV):
                    nw = min(512, VS - nv * 512)
                    rhsb = wgt.tile([128, DT * 512], bf, tag="rhsb", name="rhsb", bufs=2)
                    for kt in range(DT):
                        nc.sync.dma_start(
                            rhsb[:, kt * 512 : kt * 512 + nw],
                            d_embT[kt * 128 : (kt + 1) * 128, nv * 512 : nv * 512 + nw],
                        )
                    for mt in range(8):
                        psl = psum.tile([128, 512], f32, tag=f"ps_y{mt % 4}", bufs=1, name="ps_l")
                        for kt in range(DT):
                            nc.tensor.matmul(
                                psl[:, :nw],
                                lhsT=ob[:, kt * T + mt * 128 : kt * T + (mt + 1) * 128],
                                rhs=rhsb[:, kt * 512 : kt * 512 + nw],
                                start=(kt == 0),
                                stop=(kt == DT - 1),
                            )
                        lsb = lay.tile([128, 512], bf, tag="lsb", name="lsb", bufs=2)
                        if (nv * 8 + mt) % 2 == 0:
                            nc.scalar.activation(
                                lsb[:, :nw], psl[:, :nw], Act.Copy, scale=s_colT[:, mt : mt + 1]
                            )
                        else:
                            nc.vector.tensor_scalar(
                                lsb[:, :nw], psl[:, :nw], s_colT[:, mt : mt + 1], None, Alu.mult
                            )
                        nc.sync.dma_start(
                            d_out[mt * 128 : (mt + 1) * 128, nv * 512 : nv * 512 + nw],
                            lsb[:, :nw],
                        )

    nc.finalize()
    _prog_cache[key] = nc
    return nc


def prep_inputs(inputs):
    """Host-side: full model inputs -> list of 8 per-core in_maps."""
    idx = np.asarray(inputs["idx"])
    emb = _f32(inputs["emb"])
    pos = _f32(inputs["pos"])[0, :T]  # [T, D]
    we = _f32(inputs["emb_norm_w"])
    ts = _f32(inputs["token_shift"])
    mgw = _f32(inputs["mem_gate_w"])
    mgb = _f32(inputs["mem_gate_b"])
    memp = _f32(inputs["memory_p"])
    fnw = _f32(inputs["final_norm_w"])

    pos_rs = np.ascontiguousarray(
        pos.reshape(8, 128, D).transpose(1, 0, 2).reshape(128, 8 * D)
    ).astype(BF16)
    cst = np.zeros((128, 64), np.float32)
    cst[:, 0:4] = _cols(we)
    cst[:, 4:8] = _cols(ts * we)
    cst[:, 8:12] = _cols((1.0 - ts) * we)
    cst[:, 12:16] = _cols(0.5 * mgb)
    rc = (MIX_W / np.arange(1, T + 1, dtype=np.float32))[None, :]
    rc_bc = np.ascontiguousarray(np.broadcast_to(rc, (128, T))).astype(BF16)
    mgwT = np.ascontiguousarray(mgw.T).astype(BF16)

    stack_in = {}
    for s in ("up", "dn"):
        nw = _f32(inputs[f"{s}_norm_w"])  # [NL, D]
        cw = _f32(inputs[f"{s}_conv_w"])[:, :, 0, :]  # [NL, D, K]
        cb = _f32(inputs[f"{s}_conv_b"])  # [NL, D]
        w1 = _f32(inputs[f"{s}_w1"])
        w2 = _f32(inputs[f"{s}_w2"])
        w3 = _f32(inputs[f"{s}_w3"])
        kp = _f32(inputs[f"{s}_kp"])
        ki = _f32(inputs[f"{s}_ki"])
        gn = _f32(inputs[f"{s}_gnorm"])
        cwp = np.zeros((NL, 128, DT * 16), np.float32)
        for li in range(NL):
            cwf = cw[li] * nw[li][:, None]  # [D, K]
            taps = cwf[:, ::-1]  # tap m multiplies shift m*d
            cwp[li, :, :] = np.concatenate(
                [
                    np.concatenate(
                        [taps.reshape(DT, 128, KK)[dtt], cb[li].reshape(DT, 128)[dtt][:, None]],
                        axis=1,
                    )
                    for dtt in range(DT)
                ],
                axis=1,
            )
        pid = np.zeros((128, (NL - 1) * 12), np.float32)
        g2 = np.zeros((128, (NL - 1) * DT), np.float32)
        for li in range(1, NL):
            pb = (li - 1) * 12
            pid[:, pb : pb + 4] = _cols(kp[li - 1] + ki[li - 1] / li)
            pid[:, pb + 4 : pb + 8] = _cols(ki[li - 1] / li)
            pid[:, pb + 8 : pb + 12] = _cols(gn[li - 1])
            g2[:, (li - 1) * DT : li * DT] = _cols(gn[li - 1] * gn[li - 1])
        # diagonalized conv taps for the PE conv: block (dtt, m) at columns
        # (dtt*KK+m)*128 is diag(taps[dtt*128:(dtt+1)*128, m])
        cwd = np.zeros((NL, 128, DT * KK * 128), np.float32)
        rr = np.arange(128)
        for li in range(NL):
            cwf = cw[li] * nw[li][:, None]
            taps = cwf[:, ::-1].reshape(DT, 128, KK)
            for dtt in range(DT):
                for m in range(KK):
                    cwd[li, rr, (dtt * KK + m) * 128 + rr] = taps[dtt][:, m]
        stack_in[f"{s}_cw"] = np.ascontiguousarray(cwp)
        stack_in[f"{s}_cwd"] = np.ascontiguousarray(cwd).astype(BF16)
        stack_in[f"{s}_pid"] = pid
        stack_in[f"{s}_g2"] = np.ascontiguousarray(g2).astype(BF16)
        w13 = np.concatenate([w1.transpose(0, 2, 1), w3.transpose(0, 2, 1)], axis=2)
        stack_in[f"{s}_w13T"] = np.ascontiguousarray(w13).astype(BF16)
        stack_in[f"{s}_w2T"] = np.ascontiguousarray(w2.transpose(0, 2, 1)).astype(BF16)

    sgn = _f32(inputs["sg_norm"])
    sgdw = _f32(inputs["sg_down_w"])
    sgdb = _f32(inputs["sg_down_b"])
    sguw = _f32(inputs["sg_up_w"])
    sgub = _f32(inputs["sg_up_b"])
    dwT = np.stack(
        [np.ascontiguousarray(sgdw[k].T * sgn[k][:, None]) for k in range(3)]
    ).astype(BF16)
    uwT = np.stack([np.ascontiguousarray(sguw[k].T) for k in range(3)]).astype(BF16)
    sgc = np.zeros((128, 16), np.float32)
    for k in range(3):
        sgc[0:RANK, k] = sgdb[k]
        sgc[:, 4 + k * 4 : 8 + k * 4] = _cols(0.5 * sgub[k])

    embT = np.ascontiguousarray((emb.T * fnw[:, None]))  # [D, V] f32

    common = dict(
        pos_rs=pos_rs,
        cst=None,  # per-core (p_col differs)
        rc_bc=rc_bc,
        mgwT=mgwT,
        emb_tbl=emb,
        sg_dwT=dwT,
        sg_uwT=uwT,
        sg_cols=sgc,
        **stack_in,
    )

    in_maps = []
    for c in range(NCORES):
        b = c // 4
        vsh = c % 4
        m = dict(common)
        cstc = cst.copy()
        cstc[:, 16:20] = _cols(memp[b])
        m["cst"] = cstc
        m["idx_rs"] = np.ascontiguousarray(
            idx[b].astype(np.int32).reshape(8, 128).T
        )
        m["embT_sh"] = np.ascontiguousarray(embT[:, vsh * VS : (vsh + 1) * VS]).astype(BF16)
        in_maps.append(m)
    return in_maps


LAST_RESULTS = None


def kernel(**inputs):
    global LAST_RESULTS
    from concourse.bass_utils import run_bass_kernel_spmd

    nc = build_program()
    in_maps = prep_inputs(inputs)
    trace = bool(int(os.environ.get("KB_TRACE", "0")))
    res = run_bass_kernel_spmd(nc, in_maps, core_ids=list(range(NCORES)), trace=trace)
    LAST_RESULTS = res
    out = np.zeros((B, T, VOCAB), np.float32)
    for c in range(NCORES):
        b = c // 4
        vsh = c % 4
        out[b, :, vsh * VS : (vsh + 1) * VS] = res.results[c]["logits_sh"]
    return out

